# revision 55
# baseline (speedup 1.0000x reference)
"""Causal sparse (sliding-window) attention for Trainium2, 8 NeuronCores.

Sharding: tensor-parallel over heads (16 heads -> 2 per core).  Each core
computes the qkv projection for its 2 heads (w_qkv column-parallel), windowed
causal attention, and a partial output projection (w_out row-parallel).
The host sums the 8 partial outputs.

v3: all matmuls bf16 (1 cycle/row at any moving width; fp32r paid 4x on the
128-wide score/AV spans) and fine-grained interleaved emission: each
steady-state superstep overlaps the qkv projection of chunk n, attention of
superblock n-1 and the output projection of superblock n-2 at matmul
granularity, so psum rings drain (exp on ACT, rope products / norm / copies
on DVE, masks+broadcast on Pool) while the PE keeps running.  RoPE uses
rh(q .* sin) == rh(q) .* sin (the sin table is pair-repeated): the DVE
products q.*cos / q.*sin read the projection psum directly, the rotate-half
permutation matmul consumes the sin product, and one DVE add fuses the cos
part with the rotated psum -- no psum->sbuf staging copy for q/k.
Probabilities for the 8 key-blocks of a superblock land in one sbuf
slot-buffer in reversed key-block order, which makes the masked 128-col
blocks a regular stride-384 pattern: all masking is 2 strided affine_select
ops per (superblock, head) on the Pool engine (the one engine that cannot
touch psum; only is_ge survives walrus codegen, so the far mask is
k - q - 1 >= 0).  Output projection psums are copied to bf16 sbuf
alternating ACT/DVE and DMA'd; the host sums the 8 bf16 partials in fp32.
"""
import numpy as np

import concourse.bacc as bacc
import concourse.tile as tile
import concourse.mybir as mybir
from concourse.bass_utils import run_bass_kernel_spmd

F32 = mybir.dt.float32
BF16 = mybir.dt.bfloat16

D = 1024
L = 4096
HD = 64
N_CORES = 8
WINDOW = 512
ROPE_BASE = 10000.0
NSB = L // 512          # superblocks of 512 queries
NQB = L // 128          # 128-query blocks

# Slot order (7,6,5,4,3,2,1,0): the masked 128-col block of slot s sits at
# flat buffer column s*512 + (ki mod 4)*128, which for this order is
# 384 + s*384 for the diag group (s=0..3) and 2432 + (s-4)*384 for the far
# group (s=4..7) -- both regular stride-384 access patterns, so all masking
# is 2 strided DVE ops per (superblock, head).
ORDER = (7, 6, 5, 4, 3, 2, 1, 0)


def _attn_plan(sb):
    """Per-superblock key-block plan entries (abs key block, lo, hi) in
    emission order; lo/hi bound the valid query blocks (in 0..4)."""
    if sb == 0:
        return [(kb, kb, 4) for kb in (3, 2, 1, 0)]
    return [(sb * 4 - 4 + ki, max(0, ki - 4), min(3, ki) + 1) for ki in ORDER]


_TAGS = {}


def _tag(ret, label):
    try:
        _TAGS[ret.ins.name] = label
    except Exception:
        pass
    return ret


def _build_nc(phases=("qkv", "attn", "out"), iters=1):
    _TAGS.clear()
    nc = bacc.Bacc(None, target_bir_lowering=False)

    xT = nc.dram_tensor("xT", [D, L], BF16, kind="ExternalInput")
    wl = nc.dram_tensor("wl", [D, 384], BF16, kind="ExternalInput")
    wo = nc.dram_tensor("wo", [128, D], BF16, kind="ExternalInput")
    p2 = nc.dram_tensor("p2", [128, 128], BF16, kind="ExternalInput")
    cs = nc.dram_tensor("cs", [128, L], BF16, kind="ExternalInput")
    sn = nc.dram_tensor("sn", [128, L], BF16, kind="ExternalInput")
    ident = nc.dram_tensor("ident", [128, 128], BF16, kind="ExternalInput")
    onesd = nc.dram_tensor("onesd", [128, 32], BF16, kind="ExternalInput")
    po = nc.dram_tensor("po", [L, D], BF16, kind="ExternalOutput")

    xT3 = xT.rearrange("(ko ki) l -> ki ko l", ki=128)   # [128, 8, L]
    wl3 = wl.rearrange("(ko ki) m -> ki ko m", ki=128)   # [128, 8, 384]

    with tile.TileContext(nc) as tc:
        with tc.tile_pool(name="singles", bufs=1) as singles, \
             tc.tile_pool(name="work", bufs=4) as work, \
             tc.tile_pool(name="ptp", bufs=3) as ptp, \
             tc.tile_pool(name="outp", bufs=6) as outp, \
             tc.tile_pool(name="qk", bufs=2, space="PSUM") as qkp, \
             tc.tile_pool(name="ps", bufs=3, space="PSUM") as ps, \
             tc.tile_pool(name="ctxp", bufs=2, space="PSUM") as ctxp, \
             tc.tile_pool(name="pst", bufs=1, space="PSUM") as pst:

            w_sb = singles.tile([128, 8, 384], BF16)
            p2_sb = singles.tile([128, 128], BF16)
            id_sb = singles.tile([128, 128], BF16)
            wo_sb = singles.tile([128, D], BF16)
            cs_sb = singles.tile([128, L], BF16)
            sn_sb = singles.tile([128, L], BF16)

            qrot_sb = singles.tile([128, L], BF16)
            krot_sb = singles.tile([128, L], BF16)
            ctxT_sb = singles.tile([128, L], BF16)
            # v natural layout per 128-key block: [h0 v(64) | 1 | h1 v(64) | 1]
            v_sb = singles.tile([128, NQB, 130], BF16)

            # DMA priority: v-weight slice + first x chunk head the sync
            # queue so the PE starts ASAP; everything else follows.
            nc.sync.dma_start(w_sb[:, :, 256:384], wl3[:, :, 256:384])
            w_rest_dma = [(m * 128, (m + 1) * 128) for m in (0, 1)]
            nc.gpsimd.dma_start(p2_sb[:], p2[:])
            nc.gpsimd.dma_start(id_sb[:], ident[:])
            # first chunk's table slices land early; the big remainder follows
            nc.gpsimd.dma_start(cs_sb[:, 0:512], cs[:, 0:512])
            nc.gpsimd.dma_start(sn_sb[:, 0:512], sn[:, 0:512])
            nc.gpsimd.dma_start(cs_sb[:, 512:], cs[:, 512:])
            nc.gpsimd.dma_start(sn_sb[:, 512:], sn[:, 512:])

            def emit_const_dmas():
                nc.gpsimd.dma_start(wo_sb[:], wo[:])
                nc.gpsimd.dma_start(v_sb[:, :, 64:65], onesd[:, :, None])
                nc.gpsimd.dma_start(v_sb[:, :, 129:130], onesd[:, :, None])

            xts = {}

            def emit_xt_dma(n):
                if n >= NSB:
                    return
                span = slice(n * 512, (n + 1) * 512)
                xt = work.tile([128, 8, 512], BF16, tag="xt")
                nc.sync.dma_start(xt[:], xT3[:, :, span])
                xts[n] = xt

            # ---- micro-task generators -------------------------------------
            def qkv_tasks(n):
                """Yield PE micro-tasks for chunk n (projections + rope + v)."""
                span = slice(n * 512, (n + 1) * 512)
                xt = xts.pop(n)
                emit_xt_dma(n + 1)

                def proj(m, consumer):
                    def go():
                        psq = qkp.tile([128, 512], F32, tag="mm", name="psq")
                        for k8 in range(8):
                            _tag(nc.tensor.matmul(
                                psq[:], w_sb[:, k8, m * 128:(m + 1) * 128],
                                xt[:, k8, :], start=(k8 == 0), stop=(k8 == 7)),
                                f"qkvmm n{n} m{m} k{k8}")
                        consumer(psq)
                    return go

                rope_state = {}

                def rope_products(m, dst):
                    def consume(psq):
                        qc = work.tile([128, 512], BF16, tag="qc")
                        nc.vector.tensor_tensor(qc[:], psq[:], cs_sb[:, span],
                                                mybir.AluOpType.mult)
                        qs0 = work.tile([128, 512], BF16, tag="qs")
                        nc.vector.tensor_tensor(qs0[:], psq[:], sn_sb[:, span],
                                                mybir.AluOpType.mult)
                        rope_state[m] = (qc, qs0, dst)
                    return consume

                def rot(m):
                    def go():
                        qc, qs0, dst = rope_state.pop(m)
                        psr = qkp.tile([128, 512], F32, tag="mm", name="psr")
                        _tag(nc.tensor.matmul(psr[:], p2_sb[:], qs0[:],
                                         start=True, stop=True), f"rotmm n{n} m{m}")
                        nc.vector.tensor_tensor(dst[:, span], psr[:], qc[:],
                                                mybir.AluOpType.add)
                    return go

                vstate = {}

                def vcopy(psq):
                    vraw = work.tile([128, 512], BF16, tag="vraw")
                    nc.scalar.copy(vraw[:], psq[:])
                    vstate['raw'] = vraw

                def vtp():
                    vraw = vstate.pop('raw')
                    tp = pst.tile([128, 4, 128], BF16, tag="tp")
                    for j in range(4):
                        _tag(nc.tensor.transpose(tp[:, j],
                                            vraw[:, j * 128:(j + 1) * 128],
                                            id_sb[:]), f"vtp n{n} j{j}")
                    b0 = n * 4
                    nc.vector.tensor_copy(v_sb[:, b0:b0 + 4, 0:64],
                                          tp[:, :, 0:64])
                    nc.vector.tensor_copy(v_sb[:, b0:b0 + 4, 65:129],
                                          tp[:, :, 64:128])

                # v first: its ACT staging copy then has the whole step to
                # complete before the transposes, which run last.  Both rot
                # matmuls trail their projections so the DVE products have
                # interleaved PE work to hide behind.
                yield proj(2, vcopy)
                yield proj(0, rope_products(0, qrot_sb))
                yield proj(1, rope_products(1, krot_sb))
                yield rot(0)
                yield rot(1)
                yield vtp

            def attn_tasks(sb):
                """Yield PE micro-tasks for superblock sb's attention."""
                plan = _attn_plan(sb)
                n_s = len(plan)
                ctxs = [ctxp.tile([128, 512], F32, tag="ctx", name=f"ctx{h}")
                        for h in range(2)]
                ptbufs = [ptp.tile([128, 8, 512], BF16, tag="ptb",
                                   name=f"ptb{h}") for h in range(2)]

                def score(s):
                    kb, lo, hi = plan[s]
                    cspan = slice(lo * 128, hi * 128)
                    qspan = slice(sb * 512 + lo * 128, sb * 512 + hi * 128)

                    def go():
                        for h in range(2):
                            hp = slice(h * 64, (h + 1) * 64)
                            scp = ps.tile([128, 512], F32, tag="mm", name="scp")
                            _tag(nc.tensor.matmul(
                                scp[:, cspan],
                                krot_sb[hp, kb * 128:(kb + 1) * 128],
                                qrot_sb[hp, qspan],
                                start=True, stop=True,
                                tile_position=(h * 64, 0)),
                                f"scmm sb{sb} h{h} s{s}")
                            nc.scalar.activation(
                                ptbufs[h][:, s, cspan], scp[:, cspan],
                                mybir.ActivationFunctionType.Exp, scale=0.125)
                    return go

                def group_mask(h, start_col, cm, qstep, base):
                    # masked blocks at flat cols start_col + i*384, i=0..3.
                    # diag keeps k <= q:  iota = q - k     >= 0
                    # far  keeps k >  q:  iota = k - q - 1 >= 0
                    # (only is_ge is implemented in walrus codegen).
                    # Runs on Pool (sbuf-only), the one engine without psum.
                    # Per-head so AVs of h0 need not wait for h1's exps.
                    def go():
                        flat = ptbufs[h][:].rearrange("p a b -> p (a b)")
                        ap = flat[:, start_col:start_col + 4 * 384]
                        ap = ap.rearrange("p (a b) -> p a b",
                                          b=384)[:, :, 0:128]
                        nc.gpsimd.affine_select(
                            ap, ap, [[0, 4], [qstep, 128]],
                            mybir.AluOpType.is_ge, 0.0,
                            base=base, channel_multiplier=cm)
                    return go

                def av(s, h):
                    kb, lo, hi = plan[s]
                    cspan = slice(lo * 128, hi * 128)

                    def go():
                        _tag(nc.tensor.matmul(
                            ctxs[h][0:65, cspan],
                            v_sb[:, kb, h * 65:(h + 1) * 65],
                            ptbufs[h][:, s, cspan],
                            start=(s == 0), stop=(s == n_s - 1),
                            skip_group_check=True),
                            f"avmm sb{sb} h{h} s{s}")
                    return go

                def norm():
                    sspan = slice(sb * 512, (sb + 1) * 512)
                    for h in range(2):
                        hp = slice(h * 64, (h + 1) * 64)
                        rt = work.tile([1, 512], F32, tag="rt")
                        nc.vector.reciprocal(rt[:], ctxs[h][64:65, :])
                        rb = work.tile([64, 512], F32, tag="rb")
                        nc.gpsimd.partition_broadcast(rb[:], rt[:])
                        nc.vector.tensor_tensor(ctxT_sb[hp, sspan],
                                                ctxs[h][0:64, :],
                                                rb[:], mybir.AluOpType.mult)

                if sb == 0:
                    for s in range(4):
                        yield score(s)
                    yield group_mask(0, 384, -1, 1, 0)
                    yield group_mask(1, 384, -1, 1, 0)
                    for s in range(4):
                        yield av(s, 0)
                        yield av(s, 1)
                else:
                    for s in range(4):           # diag slots (in-sb key blocks)
                        yield score(s)
                    yield group_mask(0, 384, -1, 1, 0)
                    yield group_mask(1, 384, -1, 1, 0)
                    yield score(4)
                    yield av(0, 0)
                    yield av(0, 1)
                    yield score(5)
                    yield av(1, 0)
                    yield av(1, 1)
                    yield score(6)
                    yield av(2, 0)
                    yield av(2, 1)
                    yield score(7)
                    yield av(3, 0)
                    yield av(3, 1)
                    yield group_mask(0, 2432, 1, -1, -1)
                    yield group_mask(1, 2432, 1, -1, -1)
                    yield av(4, 0)
                    yield av(5, 0)
                    yield av(4, 1)
                    yield av(5, 1)
                    yield av(6, 0)
                    yield av(7, 0)
                    yield av(6, 1)
                    yield av(7, 1)
                yield norm

            def out_tasks(sb):
                """Yield PE micro-tasks for superblock sb's output projection."""
                for ti, t in enumerate(range(sb * 4, sb * 4 + 4)):
                    for nn in range(2):
                        def go(t=t, nn=nn, ti=ti):
                            op = ps.tile([128, 512], F32, tag="mm", name="op")
                            _tag(nc.tensor.matmul(
                                op[:], ctxT_sb[:, t * 128:(t + 1) * 128],
                                wo_sb[:, nn * 512:(nn + 1) * 512],
                                start=True, stop=True), f"outmm t{t} n{nn}")
                            osb = outp.tile([128, 512], BF16, tag="ob")
                            if (ti * 2 + nn) % 2 == 0:
                                nc.scalar.copy(osb[:], op[:])
                            else:
                                nc.vector.tensor_copy(osb[:], op[:])
                            nc.scalar.dma_start(
                                po[t * 128:(t + 1) * 128,
                                   nn * 512:(nn + 1) * 512], osb[:])
                        yield go

            def interleave(gens_spans):
                """Emit tasks from each generator spread over its (start, end)
                fraction of the superstep; stable order within a generator."""
                sched = []
                for g, (lo, hi) in gens_spans:
                    lst = list(g)
                    k = len(lst)
                    for i, item in enumerate(lst):
                        pos = lo + (hi - lo) * (i / max(k - 1, 1))
                        sched.append((pos, len(sched), item))
                sched.sort(key=lambda x: (x[0], x[1]))
                for _, _, item in sched:
                    item()

            def emit_body():
                emit_xt_dma(0)
                for lo_, hi_ in w_rest_dma:
                    nc.sync.dma_start(w_sb[:, :, lo_:hi_], wl3[:, :, lo_:hi_])
                del w_rest_dma[:]
                for n in range(NSB + 2):
                    gens = []
                    if n < NSB and "qkv" in phases:
                        gens.append((qkv_tasks(n), (0.0, 0.8)))
                    if 1 <= n <= NSB and "attn" in phases:
                        gens.append((attn_tasks(n - 1), (0.0, 1.0)))
                    if n >= 2 and "out" in phases:
                        gens.append((out_tasks(n - 2), (0.35, 0.95)))
                    interleave(gens)
                    if n == 0:
                        emit_const_dmas()

            if iters == 1:
                emit_body()
            else:
                with tc.For_i(0, iters, 1):
                    emit_body()
    nc.finalize()
    return nc


def _host_constants():
    import ml_dtypes
    # RoPE tables, transposed + duplicated for the two packed head halves
    inv_freq = (1.0 / (ROPE_BASE ** (np.arange(0, HD, 2, dtype=np.float32)
                                     / np.float32(HD)))).astype(np.float32)
    pos = np.arange(L, dtype=np.float32)
    freqs = pos[:, None] * inv_freq[None, :]            # [L, 32]
    cos = np.repeat(np.cos(freqs), 2, axis=-1)          # [L, 64]
    sin = np.repeat(np.sin(freqs), 2, axis=-1)
    cs = np.ascontiguousarray(np.vstack([cos.T, cos.T])).astype(
        ml_dtypes.bfloat16)                             # [128, L]
    sn = np.ascontiguousarray(np.vstack([sin.T, sin.T])).astype(
        ml_dtypes.bfloat16)

    # rotate-half as a column-space permutation: rh(q) = q @ Pc
    pc = np.zeros((HD, HD), np.float32)
    for m in range(HD // 2):
        pc[2 * m + 1, 2 * m] = -1.0
        pc[2 * m, 2 * m + 1] = 1.0
    p2 = np.zeros((128, 128), np.float32)
    p2[:64, :64] = pc
    p2[64:, 64:] = pc
    p2 = p2.astype(ml_dtypes.bfloat16)

    ident = np.eye(128, dtype=ml_dtypes.bfloat16)
    onesd = np.ones((128, 32), ml_dtypes.bfloat16)
    return cs, sn, p2, ident, onesd


_NC_CACHE = {}


def make_in_maps(x, w_qkv, w_out):
    import ml_dtypes
    x = np.asarray(x, np.float32)
    w_qkv = np.asarray(w_qkv, np.float32)
    w_out = np.asarray(w_out, np.float32)
    B = x.shape[0]
    assert x.shape == (B, L, D) and B == 1

    xT = np.ascontiguousarray(x[0].T).astype(ml_dtypes.bfloat16)   # [D, L]
    cs, sn, p2, ident, onesd = _host_constants()

    in_maps = []
    for c in range(N_CORES):
        h0 = 2 * c
        col = slice(h0 * HD, (h0 + 2) * HD)
        wl = np.ascontiguousarray(np.concatenate(
            [w_qkv[:, 0 * D:1 * D][:, col],
             w_qkv[:, 1 * D:2 * D][:, col],
             w_qkv[:, 2 * D:3 * D][:, col]],
            axis=1)).astype(ml_dtypes.bfloat16)         # [D, 384]
        wo = np.ascontiguousarray(
            w_out[h0 * HD:(h0 + 2) * HD, :]).astype(ml_dtypes.bfloat16)
        in_maps.append({"xT": xT, "wl": wl, "wo": wo, "p2": p2,
                        "cs": cs, "sn": sn,
                        "ident": ident, "onesd": onesd})
    return in_maps


def kernel(x, w_qkv, w_out):
    if "nc" not in _NC_CACHE:
        _NC_CACHE["nc"] = _build_nc()
    nc = _NC_CACHE["nc"]

    in_maps = make_in_maps(x, w_qkv, w_out)

    res = run_bass_kernel_spmd(nc, in_maps, core_ids=list(range(N_CORES)))
    out = np.zeros((L, D), np.float32)
    for r in res.results:
        out += r["po"].astype(np.float32)
    return out[None]
